# revision 12
# baseline (speedup 1.0000x reference)
"""Bass/Trainium2 kernel for CoOccurrenceSemanticGroundingLoss.

Reference computation (B=128, T=1024, V=512, L=20):
  present[b,t] = any_l(gs[b,l]==t); count=sum_b present; valid=(0<count<B)
  tgt[b,t]   = +1 if present&valid else -1
  loss[b]    = mean_{t,v} (logits[b,t,v] - tgt[b,t])^2
  entropy[b,t] = -sum_v p*log(p)

Device strategy (8 cores, SPMD):
  - Shard T across cores (128 t per core), full B=128 on partitions.
  - Host precomputes tsc = -2*tgt (tiny: derived from 10KB grounding_signal)
    so each core's kernel is a pure streaming reduction. The only
    cross-example quantity (count/valid) is folded into tsc on host.
  - Per [128(b),512(v)] tile pair:
      logits moments: sum x, sum x^2 via DVE bn_stats for 6/8 columns per
      group and via ACT Square/Copy+accum for 2/8 (engine load balance);
      entropy: ACT Ln(p), then DVE scalar_tensor_tensor (-p)*ln(p) with
      accum -> entropy column.
  - loss uses sum_v (x-t)^2 = S2 + tsc*S1 + V  (t^2==1); the +T*V and
    1/(T*V) normalization happen on host after summing per-core partials.
"""

import numpy as np

_B, _T, _V, _M = 128, 1024, 512, 8
_TS = _T // _M   # 128 t-columns per core
_G = 8           # t-columns per DMA group (2 MiB transfers)
_NG = _TS // _G  # 16 groups
_JD = 6          # per group: j < _JD stats on DVE (bn_stats), rest on ACT

_cache = {}
last_results = None  # BassKernelResults of the most recent run (for profiling)


def _get_nc():
    if "nc" in _cache:
        return _cache["nc"]
    import concourse.bacc as bacc
    import concourse.tile as tile
    from concourse import mybir

    f32 = mybir.dt.float32
    Alu = mybir.AluOpType
    Act = mybir.ActivationFunctionType
    nc = bacc.Bacc()

    X = nc.dram_tensor("x_logits", [_B, _TS, _V], f32, kind="ExternalInput")
    P = nc.dram_tensor("p_prior", [_B, _TS, _V], f32, kind="ExternalInput")
    TSC = nc.dram_tensor("tsc", [_B, _TS], f32, kind="ExternalInput")
    ENT = nc.dram_tensor("ent", [_B, _TS], f32, kind="ExternalOutput")
    LOSSP = nc.dram_tensor("lossp", [_B, 1], f32, kind="ExternalOutput")

    with tile.TileContext(nc) as tc:
        with (
            tc.tile_pool(name="io", bufs=2) as io,
            tc.tile_pool(name="scr", bufs=4) as scrp,
            tc.tile_pool(name="stage", bufs=1) as stage,
        ):
            ts_sb = stage.tile([_B, _TS], f32, tag="ts_sb")
            nc.gpsimd.dma_start(out=ts_sb, in_=TSC[:, :])
            bn_stage = stage.tile([_B, _TS, 6], f32, tag="bn_stage")
            ent_stage = stage.tile([_B, _TS], f32, tag="ent_stage")
            # ACT-path moment columns: [B, group, j-_JD] for S1=sum x, S2=sum x^2
            s1a = stage.tile([_B, _NG, _G - _JD], f32, tag="s1a")
            s2a = stage.tile([_B, _NG, _G - _JD], f32, tag="s2a")

            for g in range(_NG):
                t0 = g * _G
                xt = io.tile([_B, _G * _V], f32, tag="xt")
                nc.gpsimd.dma_start(out=xt, in_=X[:, t0 : t0 + _G, :])
                pt = io.tile([_B, _G * _V], f32, tag="pt")
                nc.gpsimd.dma_start(out=pt, in_=P[:, t0 : t0 + _G, :])
                lg = io.tile([_B, _G * _V], f32, tag="lg")
                # p comes from softmax so p >= ~3e-8 > 0 always; Ln(p) is
                # safe without an epsilon bias (bias=0.0 uses the preamble
                # const tensor -> no cross-engine dependency).
                nc.scalar.activation(out=lg, in_=pt, func=Act.Ln)
                for j in range(_G):
                    t = t0 + j
                    xs = xt[:, j * _V : (j + 1) * _V]
                    if j < _JD:
                        nc.vector.bn_stats(out=bn_stage[:, t, :], in_=xs)
                    else:
                        sq = scrp.tile([_B, _V], f32, tag="act_scr")
                        nc.scalar.activation(
                            out=sq, in_=xs, func=Act.Square,
                            accum_out=s2a[:, g, j - _JD : j - _JD + 1],
                        )
                        cp = scrp.tile([_B, _V], f32, tag="act_scr")
                        nc.scalar.activation(
                            out=cp, in_=xs, func=Act.Copy,
                            accum_out=s1a[:, g, j - _JD : j - _JD + 1],
                        )
                    scr = scrp.tile([_B, _V], f32, tag="stt_scr")
                    nc.vector.scalar_tensor_tensor(
                        out=scr,
                        in0=pt[:, j * _V : (j + 1) * _V],
                        scalar=-1.0,
                        in1=lg[:, j * _V : (j + 1) * _V],
                        op0=Alu.mult,
                        op1=Alu.mult,
                        accum_out=ent_stage[:, t : t + 1],
                    )

            # Combined per-(b,t) loss column H = tsc*S1 + S2:
            #  - DVE columns (j < _JD) from bn_stats even/odd moments:
            #    [cnt_e, m_e, cnt_e*var_e, cnt_o, m_o, cnt_o*var_o], cnt=256
            #    S1 = 256*(m_e+m_o); S2 = w_e+w_o + 256*(m_e^2+m_o^2)
            #    H = 256*(tsc*(m_e+m_o) + m_e^2 + m_o^2) + w_e + w_o
            #  - ACT columns (j >= _JD): H = tsc*S1a + S2a directly.
            bn4 = bn_stage.rearrange("b (g j) s -> b g j s", g=_NG)
            ts4 = ts_sb.rearrange("b (g j) -> b g j", g=_NG)
            H = stage.tile([_B, _NG, _G], f32, tag="fx_h")
            dcols = (slice(None), slice(None), slice(0, _JD))
            m_e = bn4[:, :, : _JD, 1]
            w_e = bn4[:, :, : _JD, 2]
            m_o = bn4[:, :, : _JD, 4]
            w_o = bn4[:, :, : _JD, 5]
            shp = [_B, _NG, _JD]
            A = stage.tile(shp, f32, tag="fx_a")
            nc.vector.tensor_add(A, m_e, m_o)
            Bv = stage.tile(shp, f32, tag="fx_b")
            nc.vector.tensor_mul(Bv, A, ts4[:, :, : _JD])
            C = stage.tile(shp, f32, tag="fx_c")
            nc.vector.tensor_mul(C, m_e, m_e)
            D = stage.tile(shp, f32, tag="fx_d")
            nc.vector.tensor_mul(D, m_o, m_o)
            E = stage.tile(shp, f32, tag="fx_e")
            nc.vector.tensor_add(E, C, D)
            F = stage.tile(shp, f32, tag="fx_f")
            nc.vector.tensor_add(F, Bv, E)
            Gv = stage.tile(shp, f32, tag="fx_g")
            nc.vector.tensor_add(Gv, w_e, w_o)
            nc.vector.scalar_tensor_tensor(
                out=H[:, :, : _JD], in0=F, scalar=256.0, in1=Gv,
                op0=Alu.mult, op1=Alu.add,
            )
            # ACT columns
            Ta = stage.tile([_B, _NG, _G - _JD], f32, tag="fx_ta")
            nc.vector.tensor_mul(Ta, s1a, ts4[:, :, _JD:])
            nc.vector.tensor_add(H[:, :, _JD:], Ta, s2a)

            lossp = stage.tile([_B, 1], f32, tag="lossp")
            nc.vector.reduce_sum(lossp, H, axis=mybir.AxisListType.XY)

            nc.sync.dma_start(out=ENT[:, :], in_=ent_stage)
            nc.sync.dma_start(out=LOSSP[:, :], in_=lossp)

    nc.compile()
    _cache["nc"] = nc
    return nc


def kernel(
    visual_features=None,
    text_features=None,
    semantic_prior=None,
    semantic_prior_logits=None,
    grounding_signal=None,
    **_unused,
):
    global last_results
    gs = np.asarray(grounding_signal).reshape(_B, -1).astype(np.int64)
    present = np.zeros((_B, _T), dtype=bool)
    present[np.arange(_B)[:, None], gs] = True
    count = present.sum(axis=0)
    valid = (count > 0) & (count < _B)
    tgt = np.where(present & valid[None, :], np.float32(1.0), np.float32(-1.0))
    tsc_full = (-2.0 * tgt).astype(np.float32)  # [B, T]

    lg = np.ascontiguousarray(np.asarray(semantic_prior_logits), dtype=np.float32)
    pr = np.ascontiguousarray(np.asarray(semantic_prior), dtype=np.float32)

    in_maps = []
    for c in range(_M):
        sl = slice(c * _TS, (c + 1) * _TS)
        in_maps.append(
            {
                "x_logits": np.ascontiguousarray(lg[:, sl, :]),
                "p_prior": np.ascontiguousarray(pr[:, sl, :]),
                "tsc": np.ascontiguousarray(tsc_full[:, sl]),
            }
        )

    from concourse.bass_utils import run_bass_kernel_spmd

    nc = _get_nc()
    last_results = run_bass_kernel_spmd(nc, in_maps, core_ids=list(range(_M)))
    res = last_results.results

    ent = np.concatenate([r["ent"] for r in res], axis=1).astype(np.float32)
    lsum = np.sum(
        np.stack([r["lossp"][:, 0] for r in res]).astype(np.float64), axis=0
    )
    tv = float(_T * _V)
    loss = ((lsum + tv) / tv).astype(np.float32)
    return loss, ent


# revision 13
# speedup vs baseline: 1.1008x; 1.1008x over previous
"""Bass/Trainium2 kernel for CoOccurrenceSemanticGroundingLoss.

Reference computation (B=128, T=1024, V=512, L=20):
  present[b,t] = any_l(gs[b,l]==t); count=sum_b present; valid=(0<count<B)
  tgt[b,t]   = +1 if present&valid else -1
  loss[b]    = mean_{t,v} (logits[b,t,v] - tgt[b,t])^2
  entropy[b,t] = -sum_v p*log(p)

Device strategy (8 cores, SPMD):
  - Shard T across cores (128 t per core), full B=128 on partitions.
  - Host precomputes tsc = -2*tgt (tiny: derived from 10KB grounding_signal)
    so each core's kernel is a pure streaming reduction; the cross-example
    count/valid mask is folded into tsc on host.
  - Entropy per [128(b),512(v)] tile: ACT Ln(p) then DVE
    scalar_tensor_tensor (-p)*ln(p) with accum -> entropy column.
  - Logits moments (S1=sum_v x, S2=sum_v x^2): DVE bn_stats for _JD
    columns per 8-wide group, ACT Square/Copy+accum for the rest
    (engine load balance). The ACT-stat columns stream through their own
    tile (xa) so DVE and ACT each gate only their own input buffers.
  - loss uses sum_v (x-t)^2 = S2 + tsc*S1 + V  (t^2==1); +T*V and the
    1/(T*V) normalization happen on host over the 8 per-core partials.
  - Final groups taper (4,2,1,1) so the compute tail after the last DMA
    byte is one t-column, not a full 8-wide group.
"""

import numpy as np

_B, _T, _V, _M = 128, 1024, 512, 8
_TS = _T // _M   # 128 t-columns per core
_GF = 8          # full-group width (2 MiB DMA per tensor)
_NGF = 15        # full groups: 120 columns
_TAIL = (4, 2, 1, 1)  # tapered tail groups: 8 columns
_JD = 5          # per full group: j < _JD stats on DVE, rest on ACT

_cache = {}
last_results = None  # BassKernelResults of the most recent run (for profiling)


def _get_nc():
    if "nc" in _cache:
        return _cache["nc"]
    import concourse.bacc as bacc
    import concourse.tile as tile
    from concourse import mybir

    f32 = mybir.dt.float32
    Alu = mybir.AluOpType
    Act = mybir.ActivationFunctionType
    nc = bacc.Bacc()

    X = nc.dram_tensor("x_logits", [_B, _TS, _V], f32, kind="ExternalInput")
    P = nc.dram_tensor("p_prior", [_B, _TS, _V], f32, kind="ExternalInput")
    TSC = nc.dram_tensor("tsc", [_B, _TS], f32, kind="ExternalInput")
    ENT = nc.dram_tensor("ent", [_B, _TS], f32, kind="ExternalOutput")
    LOSSP = nc.dram_tensor("lossp", [_B, 1], f32, kind="ExternalOutput")

    nact = _GF - _JD  # ACT-stat columns per full group

    with tile.TileContext(nc) as tc:
        with (
            tc.tile_pool(name="io", bufs=3) as io,
            tc.tile_pool(name="scr", bufs=4) as scrp,
            tc.tile_pool(name="stage", bufs=1) as stage,
        ):
            ts_sb = stage.tile([_B, _TS], f32, tag="ts_sb")
            nc.gpsimd.dma_start(out=ts_sb, in_=TSC[:, :])
            bn_stage = stage.tile([_B, _TS, 6], f32, tag="bn_stage")
            ent_stage = stage.tile([_B, _TS], f32, tag="ent_stage")
            # ACT-path moments: [B, full-group, j-_JD]
            s1a = stage.tile([_B, _NGF, nact], f32, tag="s1a")
            s2a = stage.tile([_B, _NGF, nact], f32, tag="s2a")

            def do_entropy(pt, lg, goff, t0, gw):
                for j in range(gw):
                    scr = scrp.tile([_B, _V], f32, tag="stt_scr")
                    nc.vector.scalar_tensor_tensor(
                        out=scr,
                        in0=pt[:, (goff + j) * _V : (goff + j + 1) * _V],
                        scalar=-1.0,
                        in1=lg[:, (goff + j) * _V : (goff + j + 1) * _V],
                        op0=Alu.mult,
                        op1=Alu.mult,
                        accum_out=ent_stage[:, t0 + j : t0 + j + 1],
                    )

            # 15 full groups: xt (DVE columns) + xa (ACT columns) split DMAs
            for g in range(_NGF):
                t0 = g * _GF
                xt = io.tile([_B, _JD * _V], f32, tag="xt")
                nc.gpsimd.dma_start(out=xt, in_=X[:, t0 : t0 + _JD, :])
                xa = io.tile([_B, nact * _V], f32, tag="xa")
                nc.gpsimd.dma_start(out=xa, in_=X[:, t0 + _JD : t0 + _GF, :])
                pt = io.tile([_B, _GF * _V], f32, tag="pt")
                nc.gpsimd.dma_start(out=pt, in_=P[:, t0 : t0 + _GF, :])
                lg = io.tile([_B, _GF * _V], f32, tag="lg")
                # p comes from softmax so p >= ~3e-8 > 0 always; Ln(p) is
                # safe without an epsilon bias (bias=0.0 uses the preamble
                # const tensor -> no cross-engine dependency).
                nc.scalar.activation(out=lg, in_=pt, func=Act.Ln)
                for j in range(_JD):
                    nc.vector.bn_stats(
                        out=bn_stage[:, t0 + j, :],
                        in_=xt[:, j * _V : (j + 1) * _V],
                    )
                for j in range(nact):
                    xs = xa[:, j * _V : (j + 1) * _V]
                    sq = scrp.tile([_B, _V], f32, tag="act_scr")
                    nc.scalar.activation(
                        out=sq, in_=xs, func=Act.Square,
                        accum_out=s2a[:, g, j : j + 1],
                    )
                    cp = scrp.tile([_B, _V], f32, tag="act_scr")
                    nc.scalar.activation(
                        out=cp, in_=xs, func=Act.Copy,
                        accum_out=s1a[:, g, j : j + 1],
                    )
                do_entropy(pt, lg, 0, t0, _GF)

            # tapered tail groups, stats all on DVE (minimal per-group tail)
            t0 = _NGF * _GF
            for gw in _TAIL:
                xt = io.tile([_B, gw * _V], f32, tag="xt")
                nc.gpsimd.dma_start(out=xt, in_=X[:, t0 : t0 + gw, :])
                pt = io.tile([_B, gw * _V], f32, tag="pt")
                nc.gpsimd.dma_start(out=pt, in_=P[:, t0 : t0 + gw, :])
                lg = io.tile([_B, gw * _V], f32, tag="lg")
                nc.scalar.activation(out=lg, in_=pt, func=Act.Ln)
                for j in range(gw):
                    nc.vector.bn_stats(
                        out=bn_stage[:, t0 + j, :],
                        in_=xt[:, j * _V : (j + 1) * _V],
                    )
                do_entropy(pt, lg, 0, t0, gw)
                t0 += gw

            # ---- loss column H[b,t] = tsc*S1 + S2 ----
            # bn_stats even/odd moments (cnt=256 each):
            #   S1 = 256*(m_e+m_o); S2 = w_e+w_o + 256*(m_e^2+m_o^2)
            #   H = 256*(tsc*(m_e+m_o) + m_e^2 + m_o^2) + w_e + w_o
            H = stage.tile([_B, _TS], f32, tag="fx_h")

            def bn_fixup(bn_sl, ts_sl, h_sl, shp):
                m_e = bn_sl[..., 1]
                w_e = bn_sl[..., 2]
                m_o = bn_sl[..., 4]
                w_o = bn_sl[..., 5]
                A = stage.tile(shp, f32, tag="fx_a")
                nc.vector.tensor_add(A, m_e, m_o)
                Bv = stage.tile(shp, f32, tag="fx_b")
                nc.vector.tensor_mul(Bv, A, ts_sl)
                C = stage.tile(shp, f32, tag="fx_c")
                nc.vector.tensor_mul(C, m_e, m_e)
                D = stage.tile(shp, f32, tag="fx_d")
                nc.vector.tensor_mul(D, m_o, m_o)
                E = stage.tile(shp, f32, tag="fx_e")
                nc.vector.tensor_add(E, C, D)
                F = stage.tile(shp, f32, tag="fx_f")
                nc.vector.tensor_add(F, Bv, E)
                Gv = stage.tile(shp, f32, tag="fx_g")
                nc.vector.tensor_add(Gv, w_e, w_o)
                nc.vector.scalar_tensor_tensor(
                    out=h_sl, in0=F, scalar=256.0, in1=Gv,
                    op0=Alu.mult, op1=Alu.add,
                )

            nfull = _NGF * _GF
            bn4 = bn_stage[:, :nfull, :].rearrange(
                "b (g j) s -> b g j s", g=_NGF
            )
            ts4 = ts_sb[:, :nfull].rearrange("b (g j) -> b g j", g=_NGF)
            h4 = H[:, :nfull].rearrange("b (g j) -> b g j", g=_NGF)
            bn_fixup(
                bn4[:, :, : _JD, :], ts4[:, :, : _JD], h4[:, :, : _JD],
                [_B, _NGF, _JD],
            )
            # ACT columns of full groups
            Ta = stage.tile([_B, _NGF, nact], f32, tag="fx_ta")
            nc.vector.tensor_mul(Ta, s1a, ts4[:, :, _JD:])
            nc.vector.tensor_add(h4[:, :, _JD:], Ta, s2a)
            # tail columns (all bn)
            bn_fixup(
                bn_stage[:, nfull:, :], ts_sb[:, nfull:], H[:, nfull:],
                [_B, _TS - nfull],
            )

            lossp = stage.tile([_B, 1], f32, tag="lossp")
            nc.vector.reduce_sum(lossp, H, axis=mybir.AxisListType.X)

            nc.sync.dma_start(out=ENT[:, :], in_=ent_stage)
            nc.sync.dma_start(out=LOSSP[:, :], in_=lossp)

    nc.compile()
    _cache["nc"] = nc
    return nc


def kernel(
    visual_features=None,
    text_features=None,
    semantic_prior=None,
    semantic_prior_logits=None,
    grounding_signal=None,
    **_unused,
):
    global last_results
    gs = np.asarray(grounding_signal).reshape(_B, -1).astype(np.int64)
    present = np.zeros((_B, _T), dtype=bool)
    present[np.arange(_B)[:, None], gs] = True
    count = present.sum(axis=0)
    valid = (count > 0) & (count < _B)
    tgt = np.where(present & valid[None, :], np.float32(1.0), np.float32(-1.0))
    tsc_full = (-2.0 * tgt).astype(np.float32)  # [B, T]

    lg = np.ascontiguousarray(np.asarray(semantic_prior_logits), dtype=np.float32)
    pr = np.ascontiguousarray(np.asarray(semantic_prior), dtype=np.float32)

    in_maps = []
    for c in range(_M):
        sl = slice(c * _TS, (c + 1) * _TS)
        in_maps.append(
            {
                "x_logits": np.ascontiguousarray(lg[:, sl, :]),
                "p_prior": np.ascontiguousarray(pr[:, sl, :]),
                "tsc": np.ascontiguousarray(tsc_full[:, sl]),
            }
        )

    from concourse.bass_utils import run_bass_kernel_spmd

    nc = _get_nc()
    last_results = run_bass_kernel_spmd(nc, in_maps, core_ids=list(range(_M)))
    res = last_results.results

    ent = np.concatenate([r["ent"] for r in res], axis=1).astype(np.float32)
    lsum = np.sum(
        np.stack([r["lossp"][:, 0] for r in res]).astype(np.float64), axis=0
    )
    tv = float(_T * _V)
    loss = ((lsum + tv) / tv).astype(np.float32)
    return loss, ent
